# revision 24
# baseline (speedup 1.0000x reference)
"""Trainium2 Bass kernel for the tied-weight Critic MLP (v8).

Math (derived from the reference):
  x   = concat(inputs, actions)                  (B, 420), B = 8192
  s   = sum over 30 column-blocks of 14          (B, 14)
  y1  = s @ W1.T + b1                            (B, 512)
  h1  = relu(layernorm_512(y1))        [g1=1, beta1=0, LN over the 30x tile
                                        equals LN over one 512 block]
  y2  = h1 @ (30*W2).T + b2                      (B, 512)
  h2  = relu(layernorm_512(y2))
  V   = h2 @ (30*wV).T + bV                      (B, 1)
  out = tile(V, 30)                              (B, 30)

Sharding: pure data parallelism - batch 8192 split as 1024 rows on each of
8 NeuronCores; weights replicated. The kernel emits V (128,8) per core; the
30-column broadcast is done while unsharding on the host.

v8 layout notes:
  * HWDGE DMA engine-spread rule (measured): transfers whose per-partition
    line is EXACTLY 4096B fan out across all 16 SDMA engines; other line
    sizes collapse onto a 2-engine chain (~50GB/s).
  * x is host-transposed feature-major, padded to 512 rows = 4 chunks of
    128 (row 420 = ones feeding the bias slot, 421.. = 0), and split by
    GROUP: xta holds batch tiles 0-3, xtb tiles 4-7 - both (128, 2048)
    bf16 with 4KB lines. Group 0's whole input arrives in one early DMA.
  * agent-sum on the PE: per tile 4 selector matmuls (K=128) accumulate
    into one packed PSUM [128,128] (col-groups at 32a) = the feature-major
    s-hat tile used by the LN1-stats matmuls and mm1.
  * LN1 stats ride the PE (block-diag G1); -mu rows via a tiny pbdmu
    matmul from mu4. rstd1 > 0 commutes with relu -> scales the mm1 rhs;
    mm1 emits h1 pre-normalized, feature-major -> mm2 needs no transposes.
  * Layer 2 analytically centered (W2c, b2c) -> LN2 needs only sumsq:
    ACT Square+accum_out reads y2 straight from PSUM; the relu*wv dot is
    a DVE scalar_tensor_tensor from PSUM. relu-evac of y1 split ACT/DVE.
  * The two groups are software-pipelined in ISSUE ORDER (PE executes
    strictly in order): group 1's selector/stats matmuls interleave with
    group 0's mm2 stream so the PE never idles long enough for the HAM
    to re-throttle it to 1.2GHz.
  * Per-group [128,4] f32 output DMA; junk warmup matmuls at t0.
"""

import numpy as np

N_CORES = 8
B_FULL = 8192
B_CORE = B_FULL // N_CORES  # 1024
P = 128
N_TILES = B_CORE // P  # 8
GROUP = 4  # tiles per phase group (col-group packing width)
N_GROUPS = N_TILES // GROUP
N_AGENTS = 30
IN_F = 14
K1 = IN_F + 2  # 14 s rows + ones(b1) row + (-mu) row
NFEAT = 420
HID = 512
EPS = 1e-5
N_WARMUP = 14  # junk matmuls at t0 to flip the PE HAM to 8/8
RELU_ON_ACT = (0, 2)  # tiles (within group) whose relu-evac runs on ACT

_cache = {}


def _build(bV: float):
    import concourse.tile as tile
    from concourse import bacc, mybir
    from concourse.bass import ts

    f32 = mybir.dt.float32
    bf16 = mybir.dt.bfloat16
    AF = mybir.ActivationFunctionType
    ALU = mybir.AluOpType

    nc = bacc.Bacc("TRN2")

    xta_d = nc.dram_tensor("xta", (P, 2 * B_CORE), bf16, kind="ExternalInput")
    xtb_d = nc.dram_tensor("xtb", (P, 2 * B_CORE), bf16, kind="ExternalInput")
    ssel_d = nc.dram_tensor("ssel", (P, 4 * K1), bf16, kind="ExternalInput")
    w1c_d = nc.dram_tensor("w1c", (K1, HID), bf16, kind="ExternalInput")
    statsb_d = nc.dram_tensor("statsb", (P, GROUP + P), bf16, kind="ExternalInput")
    onesbd_d = nc.dram_tensor("onesbd", (P, GROUP), f32, kind="ExternalInput")
    pbd8_d = nc.dram_tensor("pbd8", (GROUP, 2 * P), f32, kind="ExternalInput")
    w2sb_d = nc.dram_tensor("w2sb", (P, 4 * HID), bf16, kind="ExternalInput")
    b2wv_d = nc.dram_tensor("b2wv", (1, 2 * HID), bf16, kind="ExternalInput")
    out_d = nc.dram_tensor("out", (P, N_TILES), f32, kind="ExternalOutput")

    with tile.TileContext(nc) as tc:
        with (
            tc.tile_pool(name="singles", bufs=1) as singles,
            tc.tile_pool(name="s4p", bufs=2) as s4p,
            tc.tile_pool(name="gstat", bufs=2) as gstat,
            tc.tile_pool(name="hp", bufs=4) as hp,
            tc.tile_pool(name="junkp", bufs=2) as junkp,
            tc.tile_pool(name="ps1", bufs=2, space="PSUM") as ps1,
            tc.tile_pool(name="ps2", bufs=4, space="PSUM") as ps2,
            tc.tile_pool(name="pss4", bufs=1, space="PSUM") as pss4,
            tc.tile_pool(name="psst", bufs=1, space="PSUM") as psst,
        ):
            # ---- DMAs. sync ring: ssel, xta (group 0's input), w2sb.
            # scalar ring: b2wv, statsb, onesbd, pbd8, w1 strips, then xtb.
            ssel = singles.tile([P, 4 * K1], bf16)
            nc.sync.dma_start(out=ssel, in_=ssel_d[:, :])
            statsb = singles.tile([P, GROUP + P], bf16)
            nc.sync.dma_start(out=statsb, in_=statsb_d[:, :])
            xsba = singles.tile([P, 2 * B_CORE], bf16)
            nc.sync.dma_start(out=xsba, in_=xta_d[:, :])
            w2sb = singles.tile([P, 4 * HID], bf16)
            nc.sync.dma_start(out=w2sb, in_=w2sb_d[:, :])

            b2wv = singles.tile([1, 2 * HID], bf16)
            nc.scalar.dma_start(out=b2wv, in_=b2wv_d[:, :])
            xsbb = singles.tile([P, 2 * B_CORE], bf16)
            nc.scalar.dma_start(out=xsbb, in_=xtb_d[:, :])
            onesbd = singles.tile([P, GROUP], f32)
            nc.scalar.dma_start(out=onesbd, in_=onesbd_d[:, :])
            pbd8 = singles.tile([GROUP, 2 * P], f32)
            nc.scalar.dma_start(out=pbd8, in_=pbd8_d[:, :])
            # w1 block-replicated into 4 partition strips (16KB x4)
            w1rep = singles.tile([P, HID], bf16)
            for a in range(GROUP):
                nc.scalar.dma_start(
                    out=w1rep[32 * a : 32 * a + K1, :], in_=w1c_d[:, :]
                )

            def xchunk(c, t):
                src = xsba if t < GROUP else xsbb
                col = c * GROUP * P + (t % GROUP) * P
                return src[:, col : col + P]

            wm14 = statsb[:, 0:GROUP]
            g1bd = statsb[:, GROUP : GROUP + P]
            pbd = pbd8[:, 0:P]
            pbdmu = pbd8[:, P : 2 * P]
            b2c = b2wv[:, 0:HID]
            wvr = b2wv[:, HID : 2 * HID]

            # ---- constants ----
            eps128 = singles.tile([P, 1], f32)
            nc.vector.memset(eps128, EPS)
            ones1 = singles.tile([1, P], bf16)
            nc.vector.memset(ones1, 1.0)
            osb = singles.tile([P, N_TILES], f32)

            # ---- PE warmup: junk matmuls to get the HAM to K=8/8 before
            # the real pipeline starts (they only depend on one memset).
            wjunk = singles.tile([64, P], bf16)
            nc.vector.memset(wjunk, 0.125)
            warmps = ps1.tile([64, P], f32, tag="y1n")
            for _ in range(N_WARMUP):
                nc.tensor.matmul(warmps, wjunk[:, 0:64], wjunk, start=True, stop=True)

            # ---- ACT table preload (sqrt set) + wv broadcast via PE ----
            acttbl = gstat.tile([P, 1], f32, tag="acttbl")
            nc.scalar.activation(acttbl, eps128, AF.Sqrt)
            wv_ps = ps2.tile([P, HID], f32, tag="y2")
            nc.tensor.matmul(wv_ps, ones1, wvr, start=True, stop=True)
            wv_bc = singles.tile([P, HID], bf16)
            nc.scalar.copy(out=wv_bc, in_=wv_ps)

            class Grp:
                pass

            def make_group(g):
                st = Grp()
                st.g = g

                def shat():
                    st.big4 = pss4.tile([P, 2, P], f32, tag="big4", name=f"big4_{g}")
                    st4ps = st.big4[:, 0, :]
                    for ci, c in enumerate(range(4)):
                        for a in range(GROUP):
                            t = g * GROUP + a
                            nc.tensor.matmul(
                                st4ps[32 * a : 32 * a + K1, :],
                                ssel[:, c * K1 : (c + 1) * K1],
                                xchunk(c, t),
                                start=(ci == 0),
                                stop=(ci == 3),
                                tile_position=(0, 32 * a),
                            )
                    st.st4 = s4p.tile([P, P], bf16, tag="st4", name=f"st4_{g}")
                    nc.vector.tensor_copy(st.st4, st4ps)

                def stats_pre():
                    st.psb = psst.tile([P, 4, P], f32, tag="psb", name=f"psb_{g}")
                    st.mu4b = st.psb[:, 0, :]
                    gs4 = st.psb[:, 1, :]
                    st.rstdb = st.psb[:, 2, :]
                    mu4 = st.psb[0:GROUP, 3, :]
                    ey4 = st.big4[0:GROUP, 1, :]
                    nc.tensor.matmul(mu4, wm14, st.st4, start=True, stop=True)
                    nc.tensor.matmul(gs4, g1bd, st.st4, start=True, stop=True)
                    mu4sb = gstat.tile(
                        [GROUP, P], f32, tag="mu4sb", name=f"mu4sb_{g}"
                    )
                    nc.vector.tensor_copy(mu4sb, mu4)
                    nc.tensor.matmul(st.mu4b, pbdmu, mu4sb, start=True, stop=True)
                    prod = gstat.tile([P, P], f32, tag="prod", name=f"prod_{g}")
                    nc.vector.tensor_mul(prod, st.st4, gs4)
                    nc.tensor.matmul(ey4, onesbd, prod, start=True, stop=True)
                    musq1 = gstat.tile(
                        [GROUP, P], f32, tag="musq1", name=f"musq1_{g}"
                    )
                    nc.vector.tensor_mul(musq1, mu4sb, mu4sb)
                    st.var1 = gstat.tile(
                        [GROUP, P], f32, tag="var1", name=f"var1_{g}"
                    )
                    nc.vector.tensor_sub(st.var1, ey4, musq1)

                def stats_post():
                    std1 = gstat.tile([GROUP, P], f32, tag="std1", name=f"std1_{g}")
                    nc.scalar.activation(
                        std1, st.var1, AF.Sqrt, bias=eps128[0:GROUP, :], scale=1.0
                    )
                    rstd4 = gstat.tile(
                        [GROUP, P], f32, tag="rstd4", name=f"rstd4_{g}"
                    )
                    nc.vector.reciprocal_approx_fast(rstd4, std1)
                    nc.tensor.matmul(st.rstdb, pbd, rstd4, start=True, stop=True)
                    st4m = s4p.tile([P, P], bf16, tag="st4m", name=f"st4m_{g}")
                    nc.vector.tensor_add(st4m, st.st4, st.mu4b)
                    st.st4s = s4p.tile([P, P], bf16, tag="st4s", name=f"st4s_{g}")
                    nc.vector.tensor_mul(st.st4s, st4m, st.rstdb)

                st.y1ps = [None] * GROUP
                st.h1s = [None] * GROUP

                def mm1p(a0, a1):
                    # j-outer over a tile pair: consecutive matmuls hit
                    # different PE row-groups, so each LDWEIGHTS overlaps
                    # the other tile's in-flight matmul.
                    for a in (a0, a1):
                        st.y1ps[a] = ps1.tile(
                            [P, HID], f32, tag="y1n", name=f"y1n_{g}_{a}"
                        )
                    for j in range(4):
                        for a in (a0, a1):
                            nc.tensor.matmul(
                                st.y1ps[a][:, ts(j, P)],
                                w1rep[32 * a : 32 * a + K1, ts(j, P)],
                                st.st4s[32 * a : 32 * a + K1, :],
                                start=True,
                                stop=True,
                                tile_position=(32 * a, 0),
                            )

                def relu(a):
                    h1n = hp.tile([P, HID], bf16, tag="h1n", name=f"h1n_{g}_{a}")
                    if a in RELU_ON_ACT:
                        nc.scalar.activation(h1n, st.y1ps[a], AF.Relu)
                    else:
                        nc.vector.tensor_scalar(
                            out=h1n,
                            in0=st.y1ps[a],
                            scalar1=0.0,
                            scalar2=None,
                            op0=ALU.max,
                        )
                    st.h1s[a] = h1n

                def alloc_acc():
                    st.ssq4 = gstat.tile([P, GROUP], f32, tag="ssq4", name=f"ssq4_{g}")
                    st.vraw4 = gstat.tile(
                        [P, GROUP], f32, tag="vraw4", name=f"vraw4_{g}"
                    )

                def mm2t(a):
                    y2 = ps2.tile([P, HID], f32, tag="y2", name=f"y2_{g}_{a}")
                    nc.tensor.matmul(y2, ones1, b2c, start=True, stop=False)
                    for j in range(4):
                        nc.tensor.matmul(
                            y2,
                            st.h1s[a][:, ts(j, P)],
                            w2sb[:, j * HID : j * HID + HID],
                            start=False,
                            stop=(j == 3),
                        )
                    sqj = junkp.tile([P, HID], bf16, tag="sqj", name=f"sqj_{g}_{a}")
                    nc.scalar.activation(
                        sqj, y2, AF.Square, accum_out=st.ssq4[:, a : a + 1]
                    )
                    vj = junkp.tile([P, HID], bf16, tag="vj", name=f"vj_{g}_{a}")
                    nc.vector.scalar_tensor_tensor(
                        out=vj,
                        in0=y2,
                        scalar=0.0,
                        in1=wv_bc,
                        op0=ALU.max,
                        op1=ALU.mult,
                        accum_out=st.vraw4[:, a : a + 1],
                    )

                def final():
                    std4 = gstat.tile([P, GROUP], f32, tag="std4", name=f"std4_{g}")
                    nc.scalar.activation(
                        std4, st.ssq4, AF.Sqrt, bias=eps128, scale=1.0 / HID
                    )
                    rstd4v = gstat.tile(
                        [P, GROUP], f32, tag="rstd4v", name=f"rstd4v_{g}"
                    )
                    nc.vector.reciprocal_approx_fast(rstd4v, std4)
                    v4 = gstat.tile([P, GROUP], f32, tag="v4", name=f"v4_{g}")
                    nc.vector.tensor_mul(v4, st.vraw4, rstd4v)
                    nc.vector.tensor_scalar(
                        out=osb[:, g * GROUP : (g + 1) * GROUP],
                        in0=v4,
                        scalar1=float(bV),
                        scalar2=None,
                        op0=ALU.add,
                    )
                    nc.sync.dma_start(
                        out=out_d[:, g * GROUP : (g + 1) * GROUP],
                        in_=osb[:, g * GROUP : (g + 1) * GROUP],
                    )

                st.shat = shat
                st.stats_pre = stats_pre
                st.stats_post = stats_post
                st.mm1p = mm1p
                st.relu = relu
                st.alloc_acc = alloc_acc
                st.mm2t = mm2t
                st.final = final
                return st

            g0 = make_group(0)
            g1 = make_group(1)

            # ---- hand-scheduled issue order: PE is strictly in-order, so
            # group 1's matmuls are woven between group 0's to keep the PE
            # dense (HAM stays at 2.4GHz) while DVE/ACT chains overlap.
            g0.shat()
            g0.stats_pre()
            g0.stats_post()
            g0.alloc_acc()
            g1.alloc_acc()
            g0.mm1p(0, 1)
            g0.relu(0)
            g0.relu(1)
            g1.shat()
            g0.mm2t(0)
            g0.mm1p(2, 3)
            g0.relu(2)
            g1.stats_pre()
            g0.mm2t(1)
            g0.relu(3)
            g1.stats_post()
            g0.mm2t(2)
            g1.mm1p(0, 1)
            g1.relu(0)
            g0.mm2t(3)
            g1.relu(1)
            g0.final()
            g1.mm2t(0)
            g1.mm1p(2, 3)
            g1.relu(2)
            g1.mm2t(1)
            g1.relu(3)
            g1.mm2t(2)
            g1.mm2t(3)
            g1.final()

    nc.compile()
    return nc


def _prep(inputs):
    import ml_dtypes

    bf = ml_dtypes.bfloat16

    xin = np.concatenate(
        [
            np.asarray(inputs["inputs"], np.float32),
            np.asarray(inputs["actions"], np.float32),
        ],
        axis=1,
    )  # (8192, 420), column R = 14a + f
    w1 = np.asarray(inputs["w1"], np.float32)  # (512, 14)
    b1 = np.asarray(inputs["b1"], np.float32)  # (512,)
    w2 = np.asarray(inputs["w2"], np.float32)  # (512, 512)
    b2 = np.asarray(inputs["b2"], np.float32)
    wV = np.asarray(inputs["wV"], np.float32)
    bV = float(np.asarray(inputs["bV"], np.float32).reshape(-1)[0])

    # LN affine params are identity in this model; the kernel folds them away.
    for k, want in (("g1", 1.0), ("g2", 1.0), ("beta1", 0.0), ("beta2", 0.0)):
        if k in inputs:
            assert np.allclose(np.asarray(inputs[k]), want), f"{k} must be {want}"

    # x feature-major, padded to 512 rows = 4 chunks of 128:
    # rows 0..419 features, row 420 = ones, 421.. = 0.
    xT = np.ascontiguousarray(xin.T).astype(bf)  # (420, 8192)
    # selector: S[r, c*16 + f'] = 1 iff (128c + r) % 14 == f' (row < 420);
    # row 420 (chunk 3, r=36) -> bias-slot col 14.
    ssel = np.zeros((P, 4, K1), np.float32)
    for c in range(4):
        for r in range(P):
            R = P * c + r
            if R < NFEAT:
                ssel[r, c, R % IN_F] = 1.0
    ssel[NFEAT - 3 * P, 3, IN_F] = 1.0

    what = np.concatenate([w1, b1[:, None]], axis=1)  # (512, 15)
    wm1 = what.mean(axis=0)  # (15,)
    G1 = (what.T @ what) / HID  # (15, 15)

    # compact mm1 lhsT: rows 0..13 = W1.T, row 14 = b1, row 15 = ones
    w1c = np.concatenate([what.T, np.ones((1, HID), np.float32)], axis=0)  # (16,512)
    statsb = np.zeros((P, GROUP + P), np.float32)  # [wm14 | g1bd]
    onesbd = np.zeros((P, GROUP), np.float32)
    pbd8 = np.zeros((GROUP, 2 * P), np.float32)  # [pbd | pbdmu]
    for a in range(GROUP):
        o = 32 * a
        statsb[o : o + IN_F + 1, a] = wm1  # wm14
        statsb[o : o + IN_F + 1, GROUP + o : GROUP + o + IN_F + 1] = G1
        onesbd[o : o + IN_F + 1, a] = 1.0
        pbd8[a, o : o + K1] = 1.0
        pbd8[a, P + o + IN_F + 1] = -1.0  # pbdmu: -mu row slots

    # layer-2 analytic centering: y2c = h1 @ W2c + b2c has zero g-mean
    w2t = (N_AGENTS * w2).T.astype(np.float32)  # (512f, 512g)
    w2c = w2t - w2t.mean(axis=1, keepdims=True)
    b2c = (b2 - b2.mean())[None, :]
    # w2sb[p, c*512 + n] = w2c[128c + p, n]
    w2sb = np.ascontiguousarray(
        w2c.reshape(4, P, HID).transpose(1, 0, 2).reshape(P, 4 * HID)
    )
    b2wv = np.concatenate(
        [b2c, N_AGENTS * wV.reshape(1, -1)], axis=1
    )  # (1, 1024)

    common = {
        "ssel": np.ascontiguousarray(ssel.reshape(P, 4 * K1)).astype(bf),
        "w1c": w1c.astype(bf),
        "statsb": statsb.astype(bf),
        "onesbd": onesbd,
        "pbd8": pbd8,
        "w2sb": w2sb.astype(bf),
        "b2wv": np.ascontiguousarray(b2wv).astype(bf),
    }
    in_maps = []
    for core in range(N_CORES):
        blk = xT[:, core * B_CORE : (core + 1) * B_CORE]  # (420, 1024)
        xp = np.zeros((4 * P, B_CORE), dtype=bf)
        xp[:NFEAT, :] = blk
        xp[NFEAT, :] = np.float32(1.0)
        # xta: group-0 batch halves of all 4 chunks; xtb: group 1
        half = GROUP * P  # 512
        im = dict(common)
        im["xta"] = np.ascontiguousarray(
            np.concatenate(
                [xp[c * P : (c + 1) * P, 0:half] for c in range(4)], axis=1
            )
        )
        im["xtb"] = np.ascontiguousarray(
            np.concatenate(
                [xp[c * P : (c + 1) * P, half : 2 * half] for c in range(4)], axis=1
            )
        )
        in_maps.append(im)
    return in_maps, bV


def _run(inputs, trace=False):
    from concourse.bass_utils import run_bass_kernel_spmd

    in_maps, bV = _prep(inputs)
    if "nc" not in _cache:
        _cache["nc"] = _build(bV)
    res = run_bass_kernel_spmd(
        _cache["nc"], in_maps, core_ids=list(range(N_CORES)), trace=trace
    )
    # out (128, 8) per core: row p, col t -> batch row t*128 + p
    vs = [np.asarray(m["out"], np.float32).T.reshape(B_CORE, 1) for m in res.results]
    v = np.concatenate(vs, axis=0)  # (8192, 1)
    out = np.ascontiguousarray(np.tile(v, (1, N_AGENTS))).astype(np.float32)
    return out, res


def kernel(**inputs) -> np.ndarray:
    out, _ = _run(inputs, trace=False)
    return out


# revision 30
# speedup vs baseline: 1.0131x; 1.0131x over previous
"""Trainium2 Bass kernel for the tied-weight Critic MLP (v8).

Math (derived from the reference):
  x   = concat(inputs, actions)                  (B, 420), B = 8192
  s   = sum over 30 column-blocks of 14          (B, 14)
  y1  = s @ W1.T + b1                            (B, 512)
  h1  = relu(layernorm_512(y1))        [g1=1, beta1=0, LN over the 30x tile
                                        equals LN over one 512 block]
  y2  = h1 @ (30*W2).T + b2                      (B, 512)
  h2  = relu(layernorm_512(y2))
  V   = h2 @ (30*wV).T + bV                      (B, 1)
  out = tile(V, 30)                              (B, 30)

Sharding: pure data parallelism - batch 8192 split as 1024 rows on each of
8 NeuronCores; weights replicated. The kernel emits V (128,8) per core; the
30-column broadcast is done while unsharding on the host.

v8 layout notes:
  * HWDGE DMA engine-spread rule (measured): transfers whose per-partition
    line is EXACTLY 4096B fan out across all 16 SDMA engines; other line
    sizes collapse onto a 2-engine chain (~50GB/s).
  * x is host-transposed feature-major, padded to 512 rows = 4 chunks of
    128 (row 420 = ones feeding the bias slot, 421.. = 0), and split by
    GROUP: xta holds batch tiles 0-3, xtb tiles 4-7 - both (128, 2048)
    bf16 with 4KB lines. Group 0's whole input arrives in one early DMA.
  * agent-sum on the PE: per tile 4 selector matmuls (K=128) accumulate
    into one packed PSUM [128,128] (col-groups at 32a) = the feature-major
    s-hat tile used by the LN1-stats matmuls and mm1.
  * LN1 stats ride the PE (block-diag G1); -mu rows via a tiny pbdmu
    matmul from mu4. rstd1 > 0 commutes with relu -> scales the mm1 rhs;
    mm1 emits h1 pre-normalized, feature-major -> mm2 needs no transposes.
  * Layer 2 analytically centered (W2c, b2c) -> LN2 needs only sumsq:
    ACT Square+accum_out reads y2 straight from PSUM; the relu*wv dot is
    a DVE scalar_tensor_tensor from PSUM. relu-evac of y1 split ACT/DVE.
  * The two groups are software-pipelined in ISSUE ORDER (PE executes
    strictly in order): group 1's selector/stats matmuls interleave with
    group 0's mm2 stream so the PE never idles long enough for the HAM
    to re-throttle it to 1.2GHz.
  * Per-group [128,4] f32 output DMA; junk warmup matmuls at t0.
"""

import numpy as np

N_CORES = 8
B_FULL = 8192
B_CORE = B_FULL // N_CORES  # 1024
P = 128
N_TILES = B_CORE // P  # 8
GROUP = 4  # tiles per phase group (col-group packing width)
N_GROUPS = N_TILES // GROUP
N_AGENTS = 30
IN_F = 14
K1 = IN_F + 2  # 14 s rows + ones(b1) row + (-mu) row
NFEAT = 420
HID = 512
EPS = 1e-5
N_WARMUP = 14  # junk matmuls at t0 to flip the PE HAM to 8/8
RELU_ON_ACT = (0, 2)  # tiles (within group) whose relu-evac runs on ACT

_cache = {}


def _build(bV: float):
    import concourse.tile as tile
    from concourse import bacc, mybir
    from concourse.bass import ts

    f32 = mybir.dt.float32
    bf16 = mybir.dt.bfloat16
    AF = mybir.ActivationFunctionType
    ALU = mybir.AluOpType

    nc = bacc.Bacc("TRN2")

    META_COLS = 1228  # ssel 64 | statsb 132 | onesbd 8 | pbd8 512 | w1rep 512
    xta_d = nc.dram_tensor("xta", (P, 2 * B_CORE), bf16, kind="ExternalInput")
    xtb_d = nc.dram_tensor("xtb", (P, 2 * B_CORE), bf16, kind="ExternalInput")
    meta_d = nc.dram_tensor("meta", (P, META_COLS), bf16, kind="ExternalInput")
    w2sb_d = nc.dram_tensor("w2sb", (P, 4 * HID), bf16, kind="ExternalInput")
    b2wv_d = nc.dram_tensor("b2wv", (1, 2 * HID), bf16, kind="ExternalInput")
    out_d = nc.dram_tensor("out", (P, N_TILES), f32, kind="ExternalOutput")

    with tile.TileContext(nc) as tc:
        with (
            tc.tile_pool(name="singles", bufs=1) as singles,
            tc.tile_pool(name="s4p", bufs=2) as s4p,
            tc.tile_pool(name="gstat", bufs=2) as gstat,
            tc.tile_pool(name="hp", bufs=4) as hp,
            tc.tile_pool(name="junkp", bufs=2) as junkp,
            tc.tile_pool(name="ps1", bufs=2, space="PSUM") as ps1,
            tc.tile_pool(name="ps2", bufs=4, space="PSUM") as ps2,
            tc.tile_pool(name="pss4", bufs=1, space="PSUM") as pss4,
            tc.tile_pool(name="psst", bufs=1, space="PSUM") as psst,
        ):
            # ---- DMAs. The two HWDGE rings carry ONLY the three big
            # 4KB-line tensors (one issue each: issue itself costs ~0.7us
            # of engine time and transfers start only after their issue).
            # All small tensors ride ONE merged SWDGE (gpsimd) transfer on
            # the otherwise-idle third DMA path.
            meta = singles.tile([P, META_COLS], bf16)
            nc.gpsimd.dma_start(out=meta, in_=meta_d[:, :])
            b2wv = singles.tile([1, 2 * HID], bf16)
            nc.gpsimd.dma_start(out=b2wv, in_=b2wv_d[:, :])

            xsba = singles.tile([P, 2 * B_CORE], bf16)
            nc.sync.dma_start(out=xsba, in_=xta_d[:, :])
            w2sb = singles.tile([P, 4 * HID], bf16)
            nc.sync.dma_start(out=w2sb, in_=w2sb_d[:, :])
            xsbb = singles.tile([P, 2 * B_CORE], bf16)
            nc.scalar.dma_start(out=xsbb, in_=xtb_d[:, :])

            def xchunk(c, t):
                src = xsba if t < GROUP else xsbb
                col = c * GROUP * P + (t % GROUP) * P
                return src[:, col : col + P]

            ssel = meta[:, 0 : 4 * K1]
            wm14 = meta[:, 64:68]
            g1bd = meta[:, 68:196]
            onesbd = meta[:, 196:204].bitcast(f32)
            pbd = meta[0:GROUP, 204:460].bitcast(f32)
            pbdmu = meta[0:GROUP, 460:716].bitcast(f32)
            w1rep = meta[:, 716:1228]
            b2c = b2wv[:, 0:HID]
            wvr = b2wv[:, HID : 2 * HID]

            # ---- constants ----
            eps128 = singles.tile([P, 1], f32)
            nc.vector.memset(eps128, EPS)
            ones1 = singles.tile([1, P], bf16)
            nc.vector.memset(ones1, 1.0)
            osb = singles.tile([P, N_TILES], f32)

            # ---- PE warmup: junk matmuls to get the HAM to K=8/8 before
            # the real pipeline starts (they only depend on one memset).
            wjunk = singles.tile([64, P], bf16)
            nc.vector.memset(wjunk, 0.125)
            warmps = ps1.tile([64, P], f32, tag="y1n")
            for _ in range(N_WARMUP):
                nc.tensor.matmul(warmps, wjunk[:, 0:64], wjunk, start=True, stop=True)

            # ---- ACT table preload (sqrt set); wv broadcast emitted later
            # (between the stats phases) so it can't stall the early PE.
            acttbl = gstat.tile([P, 1], f32, tag="acttbl")
            nc.scalar.activation(acttbl, eps128, AF.Sqrt)
            wv_bc = singles.tile([P, HID], bf16)

            def wv_broadcast():
                wv_ps = ps2.tile([P, HID], f32, tag="y2")
                nc.tensor.matmul(wv_ps, ones1, wvr, start=True, stop=True)
                nc.scalar.copy(out=wv_bc, in_=wv_ps)

            class Grp:
                pass

            def make_group(g):
                st = Grp()
                st.g = g

                def shat():
                    st.big4 = pss4.tile([P, 2, P], f32, tag="big4", name=f"big4_{g}")
                    st4ps = st.big4[:, 0, :]
                    for ci, c in enumerate(range(4)):
                        for a in range(GROUP):
                            t = g * GROUP + a
                            nc.tensor.matmul(
                                st4ps[32 * a : 32 * a + K1, :],
                                ssel[:, c * K1 : (c + 1) * K1],
                                xchunk(c, t),
                                start=(ci == 0),
                                stop=(ci == 3),
                                tile_position=(0, 32 * a),
                            )
                    st.st4 = s4p.tile([P, P], bf16, tag="st4", name=f"st4_{g}")
                    nc.vector.tensor_copy(st.st4, st4ps)

                def stats_pre():
                    st.psb = psst.tile([P, 4, P], f32, tag="psb", name=f"psb_{g}")
                    st.mu4b = st.psb[:, 0, :]
                    gs4 = st.psb[:, 1, :]
                    st.rstdb = st.psb[:, 2, :]
                    mu4 = st.psb[0:GROUP, 3, :]
                    ey4 = st.big4[0:GROUP, 1, :]
                    nc.tensor.matmul(mu4, wm14, st.st4, start=True, stop=True)
                    nc.tensor.matmul(gs4, g1bd, st.st4, start=True, stop=True)
                    mu4sb = gstat.tile(
                        [GROUP, P], f32, tag="mu4sb", name=f"mu4sb_{g}"
                    )
                    nc.vector.tensor_copy(mu4sb, mu4)
                    nc.tensor.matmul(st.mu4b, pbdmu, mu4sb, start=True, stop=True)
                    prod = gstat.tile([P, P], f32, tag="prod", name=f"prod_{g}")
                    nc.vector.tensor_mul(prod, st.st4, gs4)
                    nc.tensor.matmul(ey4, onesbd, prod, start=True, stop=True)
                    musq1 = gstat.tile(
                        [GROUP, P], f32, tag="musq1", name=f"musq1_{g}"
                    )
                    nc.vector.tensor_mul(musq1, mu4sb, mu4sb)
                    st.var1 = gstat.tile(
                        [GROUP, P], f32, tag="var1", name=f"var1_{g}"
                    )
                    nc.vector.tensor_sub(st.var1, ey4, musq1)

                def stats_post():
                    std1 = gstat.tile([GROUP, P], f32, tag="std1", name=f"std1_{g}")
                    nc.scalar.activation(
                        std1, st.var1, AF.Sqrt, bias=eps128[0:GROUP, :], scale=1.0
                    )
                    rstd4 = gstat.tile(
                        [GROUP, P], f32, tag="rstd4", name=f"rstd4_{g}"
                    )
                    nc.vector.reciprocal_approx_fast(rstd4, std1)
                    nc.tensor.matmul(st.rstdb, pbd, rstd4, start=True, stop=True)
                    st4m = s4p.tile([P, P], bf16, tag="st4m", name=f"st4m_{g}")
                    nc.vector.tensor_add(st4m, st.st4, st.mu4b)
                    st.st4s = s4p.tile([P, P], bf16, tag="st4s", name=f"st4s_{g}")
                    nc.vector.tensor_mul(st.st4s, st4m, st.rstdb)

                st.y1ps = [None] * GROUP
                st.h1s = [None] * GROUP

                def mm1p(a0, a1):
                    # j-outer over a tile pair: consecutive matmuls hit
                    # different PE row-groups, so each LDWEIGHTS overlaps
                    # the other tile's in-flight matmul.
                    for a in (a0, a1):
                        st.y1ps[a] = ps1.tile(
                            [P, HID], f32, tag="y1n", name=f"y1n_{g}_{a}"
                        )
                    for j in range(4):
                        for a in (a0, a1):
                            nc.tensor.matmul(
                                st.y1ps[a][:, ts(j, P)],
                                w1rep[32 * a : 32 * a + K1, ts(j, P)],
                                st.st4s[32 * a : 32 * a + K1, :],
                                start=True,
                                stop=True,
                                tile_position=(32 * a, 0),
                            )

                def relu(a):
                    h1n = hp.tile([P, HID], bf16, tag="h1n", name=f"h1n_{g}_{a}")
                    if a in RELU_ON_ACT:
                        nc.scalar.activation(h1n, st.y1ps[a], AF.Relu)
                    else:
                        nc.vector.tensor_scalar(
                            out=h1n,
                            in0=st.y1ps[a],
                            scalar1=0.0,
                            scalar2=None,
                            op0=ALU.max,
                        )
                    st.h1s[a] = h1n

                def alloc_acc():
                    st.ssq4 = gstat.tile([P, GROUP], f32, tag="ssq4", name=f"ssq4_{g}")
                    st.vraw4 = gstat.tile(
                        [P, GROUP], f32, tag="vraw4", name=f"vraw4_{g}"
                    )

                def mm2t(a):
                    y2 = ps2.tile([P, HID], f32, tag="y2", name=f"y2_{g}_{a}")
                    nc.tensor.matmul(y2, ones1, b2c, start=True, stop=False)
                    for j in range(4):
                        nc.tensor.matmul(
                            y2,
                            st.h1s[a][:, ts(j, P)],
                            w2sb[:, j * HID : j * HID + HID],
                            start=False,
                            stop=(j == 3),
                        )
                    sqj = junkp.tile([P, HID], bf16, tag="sqj", name=f"sqj_{g}_{a}")
                    nc.scalar.activation(
                        sqj, y2, AF.Square, accum_out=st.ssq4[:, a : a + 1]
                    )
                    vj = junkp.tile([P, HID], bf16, tag="vj", name=f"vj_{g}_{a}")
                    nc.vector.scalar_tensor_tensor(
                        out=vj,
                        in0=y2,
                        scalar=0.0,
                        in1=wv_bc,
                        op0=ALU.max,
                        op1=ALU.mult,
                        accum_out=st.vraw4[:, a : a + 1],
                    )

                def final():
                    std4 = gstat.tile([P, GROUP], f32, tag="std4", name=f"std4_{g}")
                    nc.scalar.activation(
                        std4, st.ssq4, AF.Sqrt, bias=eps128, scale=1.0 / HID
                    )
                    rstd4v = gstat.tile(
                        [P, GROUP], f32, tag="rstd4v", name=f"rstd4v_{g}"
                    )
                    nc.vector.reciprocal_approx_fast(rstd4v, std4)
                    v4 = gstat.tile([P, GROUP], f32, tag="v4", name=f"v4_{g}")
                    nc.vector.tensor_mul(v4, st.vraw4, rstd4v)
                    nc.vector.tensor_scalar(
                        out=osb[:, g * GROUP : (g + 1) * GROUP],
                        in0=v4,
                        scalar1=float(bV),
                        scalar2=None,
                        op0=ALU.add,
                    )
                    nc.sync.dma_start(
                        out=out_d[:, g * GROUP : (g + 1) * GROUP],
                        in_=osb[:, g * GROUP : (g + 1) * GROUP],
                    )

                st.shat = shat
                st.stats_pre = stats_pre
                st.stats_post = stats_post
                st.mm1p = mm1p
                st.relu = relu
                st.alloc_acc = alloc_acc
                st.mm2t = mm2t
                st.final = final
                return st

            g0 = make_group(0)
            g1 = make_group(1)

            # ---- hand-scheduled issue order: PE is strictly in-order, so
            # group 1's matmuls are woven between group 0's to keep the PE
            # dense (HAM stays at 2.4GHz) while DVE/ACT chains overlap.
            g0.shat()
            g0.stats_pre()
            wv_broadcast()
            g0.stats_post()
            g0.alloc_acc()
            g1.alloc_acc()
            g0.mm1p(0, 1)
            g0.relu(0)
            g0.relu(1)
            g1.shat()
            g0.mm2t(0)
            g0.mm1p(2, 3)
            g0.relu(2)
            g1.stats_pre()
            g0.mm2t(1)
            g0.relu(3)
            g1.stats_post()
            g0.mm2t(2)
            g1.mm1p(0, 1)
            g1.relu(0)
            g0.mm2t(3)
            g1.relu(1)
            g0.final()
            g1.mm2t(0)
            g1.mm1p(2, 3)
            g1.relu(2)
            g1.mm2t(1)
            g1.relu(3)
            g1.mm2t(2)
            g1.mm2t(3)
            g1.final()

    nc.compile()
    return nc


def _prep(inputs):
    import ml_dtypes

    bf = ml_dtypes.bfloat16

    xin = np.concatenate(
        [
            np.asarray(inputs["inputs"], np.float32),
            np.asarray(inputs["actions"], np.float32),
        ],
        axis=1,
    )  # (8192, 420), column R = 14a + f
    w1 = np.asarray(inputs["w1"], np.float32)  # (512, 14)
    b1 = np.asarray(inputs["b1"], np.float32)  # (512,)
    w2 = np.asarray(inputs["w2"], np.float32)  # (512, 512)
    b2 = np.asarray(inputs["b2"], np.float32)
    wV = np.asarray(inputs["wV"], np.float32)
    bV = float(np.asarray(inputs["bV"], np.float32).reshape(-1)[0])

    # LN affine params are identity in this model; the kernel folds them away.
    for k, want in (("g1", 1.0), ("g2", 1.0), ("beta1", 0.0), ("beta2", 0.0)):
        if k in inputs:
            assert np.allclose(np.asarray(inputs[k]), want), f"{k} must be {want}"

    # x feature-major, padded to 512 rows = 4 chunks of 128:
    # rows 0..419 features, row 420 = ones, 421.. = 0.
    xT = np.ascontiguousarray(xin.T).astype(bf)  # (420, 8192)
    # selector: S[r, c*16 + f'] = 1 iff (128c + r) % 14 == f' (row < 420);
    # row 420 (chunk 3, r=36) -> bias-slot col 14.
    ssel = np.zeros((P, 4, K1), np.float32)
    for c in range(4):
        for r in range(P):
            R = P * c + r
            if R < NFEAT:
                ssel[r, c, R % IN_F] = 1.0
    ssel[NFEAT - 3 * P, 3, IN_F] = 1.0

    what = np.concatenate([w1, b1[:, None]], axis=1)  # (512, 15)
    wm1 = what.mean(axis=0)  # (15,)
    G1 = (what.T @ what) / HID  # (15, 15)

    # mm1 lhsT rows 0..13 = W1.T, row 14 = b1, row 15 = ones; replicated
    # at the 4 partition strips
    w1p = np.concatenate([what.T, np.ones((1, HID), np.float32)], axis=0)  # (16,512)
    w1rep = np.zeros((P, HID), np.float32)
    statsb = np.zeros((P, GROUP + P), np.float32)  # [wm14 | g1bd]
    onesbd = np.zeros((P, GROUP), np.float32)
    pbd8 = np.zeros((GROUP, 2 * P), np.float32)  # [pbd | pbdmu]
    for a in range(GROUP):
        o = 32 * a
        w1rep[o : o + K1, :] = w1p
        statsb[o : o + IN_F + 1, a] = wm1  # wm14
        statsb[o : o + IN_F + 1, GROUP + o : GROUP + o + IN_F + 1] = G1
        onesbd[o : o + IN_F + 1, a] = 1.0
        pbd8[a, o : o + K1] = 1.0
        pbd8[a, P + o + IN_F + 1] = -1.0  # pbdmu: -mu row slots

    # merged small-tensor transfer: [ssel | statsb | onesbd | pbd8 | w1rep]
    meta = np.zeros((P, 1228), dtype=bf)
    meta[:, 0:64] = np.ascontiguousarray(ssel.reshape(P, 4 * K1)).astype(bf)
    meta[:, 64:196] = statsb.astype(bf)
    meta[:, 196:204] = onesbd.astype(np.float32).view(np.uint16).view(bf)
    meta[0:GROUP, 204:716] = (
        pbd8.astype(np.float32).view(np.uint16).view(bf)
    )
    meta[:, 716:1228] = w1rep.astype(bf)

    # layer-2 analytic centering: y2c = h1 @ W2c + b2c has zero g-mean
    w2t = (N_AGENTS * w2).T.astype(np.float32)  # (512f, 512g)
    w2c = w2t - w2t.mean(axis=1, keepdims=True)
    b2c = (b2 - b2.mean())[None, :]
    # w2sb[p, c*512 + n] = w2c[128c + p, n]
    w2sb = np.ascontiguousarray(
        w2c.reshape(4, P, HID).transpose(1, 0, 2).reshape(P, 4 * HID)
    )
    b2wv = np.concatenate(
        [b2c, N_AGENTS * wV.reshape(1, -1)], axis=1
    )  # (1, 1024)

    common = {
        "meta": meta,
        "w2sb": w2sb.astype(bf),
        "b2wv": np.ascontiguousarray(b2wv).astype(bf),
    }
    in_maps = []
    for core in range(N_CORES):
        blk = xT[:, core * B_CORE : (core + 1) * B_CORE]  # (420, 1024)
        xp = np.zeros((4 * P, B_CORE), dtype=bf)
        xp[:NFEAT, :] = blk
        xp[NFEAT, :] = np.float32(1.0)
        # xta: group-0 batch halves of all 4 chunks; xtb: group 1
        half = GROUP * P  # 512
        im = dict(common)
        im["xta"] = np.ascontiguousarray(
            np.concatenate(
                [xp[c * P : (c + 1) * P, 0:half] for c in range(4)], axis=1
            )
        )
        im["xtb"] = np.ascontiguousarray(
            np.concatenate(
                [xp[c * P : (c + 1) * P, half : 2 * half] for c in range(4)], axis=1
            )
        )
        in_maps.append(im)
    return in_maps, bV


def _run(inputs, trace=False):
    from concourse.bass_utils import run_bass_kernel_spmd

    in_maps, bV = _prep(inputs)
    if "nc" not in _cache:
        _cache["nc"] = _build(bV)
    res = run_bass_kernel_spmd(
        _cache["nc"], in_maps, core_ids=list(range(N_CORES)), trace=trace
    )
    # out (128, 8) per core: row p, col t -> batch row t*128 + p
    vs = [np.asarray(m["out"], np.float32).T.reshape(B_CORE, 1) for m in res.results]
    v = np.concatenate(vs, axis=0)  # (8192, 1)
    out = np.ascontiguousarray(np.tile(v, (1, N_AGENTS))).astype(np.float32)
    return out, res


def kernel(**inputs) -> np.ndarray:
    out, _ = _run(inputs, trace=False)
    return out
